# revision 18
# baseline (speedup 1.0000x reference)
"""Trainium2 Bass kernel for nn_Block_32762010534337 (dense transformer block).

Strategy: stride-4 interleaved sequence parallel over 8 cores (core c owns
tokens {4i + c%4} of batch c//4); every core runs an identical causal
attention program. Main deviations from a plain bf16 kernel:

- QKV + wo projections run in fp8e4m3 with MatmulPerfMode.DoubleRow (two
  128-contraction subtiles per PE instruction). Weights are host-quantized
  at 64x scale; the rmsnorms are scale-invariant so Q/K dequantize for
  free, V folds 1/64 into its norm-row exp bias, and wo's 512x composite
  scale is removed in the PSUM evacuation.
- Attention: causal masking is an additive -30 matmul into the score PSUM
  (identity lhsT x host mask-bias tile), exp() writes fp8 p-tiles directly
  (score shift -ln16 keeps the range in fp8), the PV and denominator
  matmuls use DoubleRow over key-tile pairs, and the denominator
  reciprocal runs on the DVE.
- V is AllGathered in fp8 (and first), K in bf16; latency-critical
  collective stores/loads ride the SWDGE (gpsimd) DMA queue so they never
  sit behind weight-slab streaming on the sync HWDGE ring.
- One ACT table set (natural_log_exp_and_others) is pinned for the whole
  kernel - the stock per-function chooser thrashes exp<->ln table loads.

MLP stays bf16 (fp8 would blow the 2e-2 error budget through relu^2).
"""
import sys
import os

if "/opt/trn_rl_repo" not in sys.path:
    sys.path.insert(0, "/opt/trn_rl_repo")

import numpy as np

B, T, C = 2, 2048, 2048
NH, NKV, HD = 16, 4, 128
DFF = 4 * C
TQ = 512          # tokens per core (stride-4 interleaved)
CH = 256          # query tile
NT = C // 128     # 16 feature tiles
NF = DFF // 128   # 64 ff tiles
EPS = 1.1920929e-07
NCORES = 8
SW = 64.0                    # fp8 weight pre-scale
NLN16 = -2.772588722239781   # -ln 16: p8 = exp(s)/16
NLN64 = -4.1588830833596715  # -ln 64: v norm-row descale

_CACHE = None
_ACT_PATCHED = False


def _pin_act_tables():
    """Make every activation resolve to natural_log_exp_and_others (it
    contains exp, ln, relu, copy, identity, square) so the kernel pays one
    ACT table load instead of thrashing exp<->ln sets."""
    global _ACT_PATCHED
    if _ACT_PATCHED:
        return
    from concourse import bacc
    orig = bacc.get_activation_tables

    def single_set(arch):
        t = orig(arch)
        return {k: (v if k == "natural_log_exp_and_others" else set())
                for k, v in t.items()}

    bacc.get_activation_tables = single_set
    _ACT_PATCHED = True


def _build():
    import concourse.bass as bass
    import concourse.tile as tile
    from concourse import mybir, bacc

    _pin_act_tables()

    dt = mybir.dt
    f32, bf16, fp8 = dt.float32, dt.bfloat16, dt.float8e4
    Alu = mybir.AluOpType
    Act = mybir.ActivationFunctionType
    DR = mybir.MatmulPerfMode.DoubleRow

    nc = bacc.Bacc("TRN2", target_bir_lowering=False, debug=False, num_devices=NCORES)

    for val in (EPS, HD * EPS, NLN16, NLN64):
        tns = nc.alloc_sbuf_tensor(f"const-f32-{val}", [128, 1], f32)
        nc.gpsimd.memset(tns.ap(), val)
        nc.const_aps.aps[(f32, val)] = tns.ap()
    nc.all_engine_barrier()

    # host-pretiled inputs ([128, i, t] partition-major)
    x8p = nc.declare_dram_parameter("x8p", [128, NT * TQ], fp8, isOutput=False)
    xp = nc.declare_dram_parameter("xp", [128, NT * TQ], bf16, isOutput=False)
    csc = nc.declare_dram_parameter("csc", [128, TQ], bf16, isOutput=False)
    css = nc.declare_dram_parameter("css", [128, TQ], bf16, isOutput=False)
    mbias = nc.declare_dram_parameter("mbias", [128, 8 * CH], bf16, isOutput=False)
    ident = nc.declare_dram_parameter("ident", [128, 128], bf16, isOutput=False)
    # weights host-pretiled (see _prep_weights); q/k/v/o fp8 at 64x
    wq = nc.declare_dram_parameter("wq", [128, 4 * NT * TQ], fp8, isOutput=False)
    wk = nc.declare_dram_parameter("wk", [128, NT * TQ], fp8, isOutput=False)
    wv = nc.declare_dram_parameter("wv", [128, NT * TQ], fp8, isOutput=False)
    wo = nc.declare_dram_parameter("wo", [128, 4 * NT * TQ], fp8, isOutput=False)
    wfc = nc.declare_dram_parameter("wfc", [128, 16 * NT * TQ], bf16, isOutput=False)
    wproj = nc.declare_dram_parameter("wproj", [128, NT * NF * 128], bf16,
                                      isOutput=False)
    out_fm = nc.declare_dram_parameter("out", [C, TQ], f32, isOutput=True)

    ck_in = nc.dram_tensor("ck_in", [512, TQ], fp8)
    ck_out = nc.dram_tensor("ck_out", [2048, TQ], fp8)
    cv_in = nc.dram_tensor("cv_in", [512, TQ], fp8)
    cv_out = nc.dram_tensor("cv_out", [2048, TQ], fp8)

    with tile.TileContext(nc, num_cores=NCORES) as tc:
        with (
            tc.tile_pool(name="const", bufs=1) as constp,
            tc.tile_pool(name="persist", bufs=1) as pp,
            tc.tile_pool(name="work", bufs=3) as wpool,
            tc.tile_pool(name="wstream", bufs=3) as wsp,
        ):
            ones = constp.tile([128, 1], bf16, tag="ones")
            nc.gpsimd.memset(ones, 1.0)
            onesf = constp.tile([128, 1], f32, tag="onesf")
            nc.gpsimd.memset(onesf, 1.0)
            # den lhsT: value 1/8 folds the y rescale (yT = 8*y) for free
            ones2 = constp.tile([128, 2, 32], fp8, tag="ones2")
            nc.gpsimd.memset(ones2, 0.125)

            # x_mid^T lives across attention + MLP
            xmT = pp.tile([128, NT, TQ], f32, tag="xmT")

            def norm_row(ssq_ps, scale, bias, n, nb=128):
                """[1,n] psum sum-of-squares -> [nb,n] f32 bcast of
                (scale*x+bias)^(-1/2), via exp(-0.5*ln(.))."""
                ln = wpool.tile([1, n], f32, tag="srow", bufs=3, name="lnrow")
                nc.scalar.activation(ln[:], ssq_ps[:], Act.Ln, bias=bias,
                                     scale=scale)
                rs = wpool.tile([1, n], f32, tag="srow", bufs=3, name="rsrow")
                nc.scalar.activation(rs[:], ln[:], Act.Exp, scale=-0.5)
                sb = wpool.tile([nb, n], f32, tag="sbcast", bufs=3)
                nc.gpsimd.partition_broadcast(sb[:], rs[:])
                return sb

            def wslab2(param, base, width, n_i, name, wdt, pool=None,
                       tag="wslab", bufs=3):
                """Stream [128, n_i, width] weights as two half-slabs."""
                pool = pool or wsp
                half = n_i // 2
                tiles = []
                for hh in range(2):
                    ts = pool.tile([128, half, width], wdt, tag=tag,
                                   bufs=bufs, name=f"{name}_{hh}")
                    o = base + half * width * hh
                    nc.sync.dma_start(
                        ts[:],
                        param[:, o:o + half * width].rearrange(
                            "p (g t) -> p g t", t=width))
                    tiles.append(ts)

                def get(i, c0=None, c1=None, pair=False):
                    t, j = tiles[i // half], i % half
                    sl = slice(j, j + 2) if pair else j
                    return t[:, sl] if c0 is None else t[:, sl, c0:c1]
                return get

            with tc.tile_pool(name="main", bufs=1) as mp:
                xin = mp.tile([128, NT, TQ], bf16, tag="xin")
                qs_sb = mp.tile([128, NH, TQ], bf16, tag="qs_sb")
                yT8 = mp.tile([128, NH, TQ], fp8, tag="yT8")
                ident_att = mp.tile([128, 128], bf16, tag="ident")
                nc.sync.dma_start(ident_att[:], ident[:])

                qp_cm = tc.tile_pool(name="qkvtmp", bufs=1)
                qp = qp_cm.__enter__()
                x8 = qp.tile([128, NT, TQ], fp8, tag="x8")
                # quarter-granular x8 + wk loads so the first K matmuls
                # start ~4us in
                wk_tiles = []
                for qq4 in range(4):
                    nc.sync.dma_start(
                        x8[:, 4 * qq4:4 * (qq4 + 1)],
                        x8p[:, 4 * qq4 * TQ:4 * (qq4 + 1) * TQ].rearrange(
                            "p (i t) -> p i t", t=TQ))
                    ts = qp.tile([128, 4, TQ], fp8, tag="wq8", bufs=5,
                                 name=f"wk_{qq4}")
                    nc.sync.dma_start(
                        ts[:],
                        wk[:, 4 * qq4 * TQ:4 * (qq4 + 1) * TQ].rearrange(
                            "p (g t) -> p g t", t=TQ))
                    wk_tiles.append(ts)

                def wkf(i, c0=None, c1=None, pair=False):
                    t, j = wk_tiles[i // 4], i % 4
                    sl = slice(j, j + 2) if pair else j
                    return t[:, sl] if c0 is None else t[:, sl, c0:c1]
                nc.sync.dma_start(
                    xin[:], xp[:].rearrange("p (i t) -> p i t", t=TQ))
                csc_sb = qp.tile([128, TQ], bf16, tag="csc")
                nc.sync.dma_start(csc_sb[:], csc[:])
                css_sb = qp.tile([128, TQ], bf16, tag="css")
                nc.sync.dma_start(css_sb[:], css[:])
                wvf = wslab2(wv, 0, TQ, NT, "wv", fp8, pool=qp, tag="wq8",
                             bufs=5)

                def rope_copy(ps):
                    """Evacuate a QKV psum tile to SBUF (frees the bank)."""
                    raw = qp.tile([128, TQ], bf16, tag="rraw", bufs=5,
                                  name="rraw")
                    nc.scalar.copy(raw[:], ps[:])
                    return raw

                def rope_rest(raw):
                    """bf16 raw head -> rope'd bf16 tile."""
                    sw = qp.tile([128, TQ], bf16, tag="rsw", bufs=4,
                                 name="rsw")
                    nc.scalar.dma_start(sw[0:64, :], raw[64:128, :])
                    nc.scalar.dma_start(sw[64:128, :], raw[0:64, :])
                    rr = qp.tile([128, TQ], bf16, tag="rr", bufs=4,
                                 name="rr")
                    nc.vector.tensor_tensor(rr[:], raw[:], csc_sb[:], Alu.mult)
                    t2 = qp.tile([128, TQ], bf16, tag="rt2", bufs=4,
                                 name="rt2")
                    nc.vector.tensor_tensor(t2[:], sw[:], css_sb[:], Alu.mult)
                    nc.vector.tensor_tensor(rr[:], rr[:], t2[:], Alu.add)
                    return rr

                def sumsq(rr):
                    sq = qp.tile([128, TQ], bf16, tag="rsq", bufs=4,
                                 name="rsq")
                    nc.vector.tensor_tensor(sq[:], rr[:], rr[:], Alu.mult)
                    return sq

                with tc.tile_pool(name="psA", bufs=1, space="PSUM") as psA:
                    # ---- K heads: project (fp8 DR) head-sequential so head
                    # 0's norm chain starts while head 1 is still projecting.
                    # Q/K project RAW x (head-rmsnorm cancels the pre-norm
                    # scale AND the 64x fp8 weight scale exactly).
                    kraw_k = []
                    for kh in range(4):
                        kps = psA.tile([128, TQ], f32, tag="qkv", bufs=4,
                                       name=f"kps_{kh}")
                        for ip in range(0, NT, 2):
                            nc.tensor.matmul(kps[:],
                                             lhsT=wkf(ip, 128 * kh,
                                                      128 * (kh + 1),
                                                      pair=True),
                                             rhs=x8[:, ip:ip + 2, :],
                                             start=(ip == 0),
                                             stop=(ip == NT - 2),
                                             perf_mode=DR)
                        kraw_k.append(rope_copy(kps))

                    # xsq/ssq first: frees the DVE before the K rope chains
                    ssq_ps = psA.tile([1, TQ], f32, tag="row", bufs=3)
                    for i in range(NT):
                        xsq = wpool.tile([128, TQ], bf16, tag="xsq", bufs=6)
                        nc.vector.tensor_tensor(xsq[:], xin[:, i], xin[:, i],
                                                Alu.mult)
                        nc.tensor.matmul(ssq_ps[:], lhsT=ones[:], rhs=xsq[:],
                                         start=(i == 0), stop=(i == NT - 1))

                    # pre-attention rmsnorm row (for V scaling only)
                    s1ln = wpool.tile([1, TQ], f32, tag="srow", bufs=3,
                                      name="s1ln")
                    nc.scalar.activation(s1ln[:], ssq_ps[:], Act.Ln, bias=EPS,
                                         scale=1.0 / C)
                    # extra -ln64 kills the 64x fp8 weight scale on V
                    s1rs = wpool.tile([1, TQ], f32, tag="srow", bufs=3,
                                      name="s1rs")
                    nc.scalar.activation(s1rs[:], s1ln[:], Act.Exp, scale=-0.5,
                                         bias=NLN64)
                    # transpose the rsqrt row to per-token columns (PE)
                    s1c = psA.tile([128, 4], f32, tag="scol", bufs=1)
                    for t in range(4):
                        nc.tensor.transpose(s1c[:, t:t + 1],
                                            s1rs[0:1, 128 * t:128 * (t + 1)],
                                            onesf[0:1, 0:1])

                    rrs_k = []
                    sqs_k = []
                    for kh in range(4):
                        rr = rope_rest(kraw_k[kh])
                        rrs_k.append(rr)
                        sqs_k.append(sumsq(rr))
                    # per-head k-norm rows -> fp8 AllGather (FIRST)
                    for kh in range(4):
                        sps = psA.tile([1, TQ], f32, tag="row", bufs=3)
                        nc.tensor.matmul(sps[:], lhsT=ones[:], rhs=sqs_k[kh][:],
                                         start=True, stop=True)
                        klr = wpool.tile([1, TQ], f32, tag="srow", bufs=3,
                                         name=f"kln{kh}")
                        nc.scalar.activation(klr[:], sps[:], Act.Ln, bias=EPS,
                                             scale=1.0 / HD)
                        krr = wpool.tile([1, TQ], f32, tag="srow", bufs=3,
                                         name=f"krs{kh}")
                        nc.scalar.activation(krr[:], klr[:], Act.Exp,
                                             scale=-0.5)
                        sb = wpool.tile([128, TQ], f32, tag="sbcast", bufs=3)
                        nc.gpsimd.partition_broadcast(sb[:], krr[:])
                        kt = qp.tile([128, TQ], fp8, tag="ktile", bufs=2,
                                     name="kt")
                        nc.vector.tensor_tensor(kt[:], rrs_k[kh][:], sb[:],
                                                Alu.mult)
                        nc.gpsimd.dma_start(ck_in[128 * kh:128 * (kh + 1), :],
                                            kt[:])
                    nc.gpsimd.collective_compute(
                        "AllGather", Alu.bypass,
                        replica_groups=[[0, 1, 2, 3], [4, 5, 6, 7]],
                        ins=[ck_in[:]], outs=[ck_out[:]])

                    # ---- V heads (fp8 DR): token-major; per-token pre-norm
                    # scale (1/64 folded in); stored fp8, gathered FIRST ----
                    for t in range(4):
                        vps = psA.tile([128, TQ], f32, tag="qkv", bufs=4,
                                       name=f"vps_{t}")
                        for ip in range(0, NT, 2):
                            nc.tensor.matmul(
                                vps[:],
                                lhsT=x8[:, ip:ip + 2, 128 * t:128 * (t + 1)],
                                rhs=wvf(ip, pair=True),
                                start=(ip == 0), stop=(ip == NT - 2),
                                perf_mode=DR)
                        vb = wpool.tile([128, TQ], fp8, tag="vb", bufs=2)
                        nc.vector.tensor_scalar_mul(vb[:], vps[:], s1c[:, t:t + 1])
                        nc.gpsimd.dma_start(
                            cv_in[128 * t:128 * (t + 1), :], vb[:])
                    nc.gpsimd.collective_compute(
                        "AllGather", Alu.bypass,
                        replica_groups=[[0, 1, 2, 3], [4, 5, 6, 7]],
                        ins=[cv_in[:]], outs=[cv_out[:]])

                    # ---- Q heads (fp8 DR): project + rope, then norm rows
                    for hg in range(4):
                        wqf = wslab2(wq, NT * TQ * hg, TQ, NT, f"wq{hg}", fp8,
                                     pool=qp, tag="wq8", bufs=5)
                        rrs = []
                        sqs = []
                        for k in range(4):
                            qps = psA.tile([128, TQ], f32, tag="qkv", bufs=4,
                                           name=f"qps{hg}_{k}")
                            for ip in range(0, NT, 2):
                                nc.tensor.matmul(
                                    qps[:],
                                    lhsT=wqf(ip, 128 * k, 128 * (k + 1),
                                             pair=True),
                                    rhs=x8[:, ip:ip + 2, :],
                                    start=(ip == 0), stop=(ip == NT - 2),
                                    perf_mode=DR)
                            rrs.append(rope_rest(rope_copy(qps)))
                            sqs.append(sumsq(rrs[k]))
                        for k in range(4):
                            sps = psA.tile([1, TQ], f32, tag="row", bufs=3)
                            nc.tensor.matmul(sps[:], lhsT=ones[:],
                                             rhs=sqs[k][:],
                                             start=True, stop=True)
                            qlr = wpool.tile([1, TQ], f32, tag="srow",
                                             bufs=3, name=f"qln{hg}_{k}")
                            nc.scalar.activation(qlr[:], sps[:], Act.Ln,
                                                 bias=HD * EPS, scale=1.0)
                            qrr = wpool.tile([1, TQ], f32, tag="srow",
                                             bufs=3, name=f"qrs{hg}_{k}")
                            nc.scalar.activation(qrr[:], qlr[:], Act.Exp,
                                                 scale=-0.5)
                            sb = wpool.tile([128, TQ], f32, tag="sbcast",
                                            bufs=3)
                            nc.gpsimd.partition_broadcast(sb[:], qrr[:])
                            nc.vector.tensor_tensor(qs_sb[:, 4 * hg + k],
                                                    rrs[k][:], sb[:], Alu.mult)
                qp_cm.__exit__(None, None, None)

                # ---- attention phase pool: gathered K/V + p tiles ----
                ap_cm = tc.tile_pool(name="attntmp", bufs=1)
                ap = ap_cm.__enter__()
                k_sb = ap.tile([128, 4, NKV, TQ], fp8, tag="k_sb")
                v8_sb = ap.tile([128, 4, 4, NKV * HD], fp8, tag="v8_sb")
                mb_sb = ap.tile([128, 8, CH], bf16, tag="mb_sb")
                nc.sync.dma_start(mb_sb[:],
                                  mbias.rearrange("p (d q) -> p d q", q=CH))
                # loads fire the moment each AllGather lands (SWDGE queue)
                for gp in range(4):
                    nc.gpsimd.dma_start(
                        k_sb[:, gp],
                        ck_out[512 * gp:512 * (gp + 1), :].rearrange(
                            "(kh p) t -> p kh t", p=128))
                for gp in range(4):
                    nc.gpsimd.dma_start(
                        v8_sb[:, gp],
                        cv_out[512 * gp:512 * (gp + 1), :].rearrange(
                            "(cb p) f -> p cb f", p=128))

                # ---- attention: fp8 p, DoubleRow y/den, DVE recip den ----
                with tc.tile_pool(name="psB", bufs=1, space="PSUM") as psB:

                    def sc_exp(kh, j, qt, grp, tag, bufs):
                        h = 4 * kh + j
                        dlo = 8 * qt
                        sc = psB.tile([128, 4, CH], f32, tag="sc", bufs=2)
                        for mi in range(4):
                            m = 4 * grp + mi
                            gp, cb = m % 4, m // 4
                            band = m >= dlo
                            nc.tensor.matmul(
                                sc[:, mi],
                                lhsT=k_sb[:, gp, kh,
                                          128 * cb:128 * (cb + 1)],
                                rhs=qs_sb[:, h, CH * qt:CH * (qt + 1)],
                                start=True, stop=not band)
                            if band:
                                # -30 into causally-dead entries
                                nc.tensor.matmul(
                                    sc[:, mi], lhsT=ident_att[:],
                                    rhs=mb_sb[:, m - dlo],
                                    start=False, stop=True)
                        p8 = ap.tile([128, 4, CH], fp8, tag=tag, bufs=bufs)
                        nc.scalar.activation(p8[:], sc[:], Act.Exp,
                                             bias=NLN16)
                        return p8

                    def y_den(kh, p8s, qt, y_ps, den_ps):
                        nk = 8 * (qt + 1)
                        for grp in range(nk // 4):
                            for mi in (0, 2):
                                m = 4 * grp + mi
                                gp, cb = m % 4, m // 4
                                nc.tensor.matmul(
                                    y_ps[:],
                                    lhsT=v8_sb[:, gp:gp + 2, cb,
                                               128 * kh:128 * (kh + 1)],
                                    rhs=p8s[grp][:, mi:mi + 2, :],
                                    start=(m == 0), stop=(m == nk - 2),
                                    perf_mode=DR)
                                nc.tensor.matmul(
                                    den_ps[:],
                                    lhsT=ones2[:, :, 0:16],
                                    rhs=p8s[grp][:, mi:mi + 2, :],
                                    start=(m == 0), stop=(m == nk - 2),
                                    perf_mode=DR)

                    # kh=0: all scores+exp first (PE never blocks on the V
                    # gather behind the first y matmul), then y/den
                    p8_kh0 = {}
                    for j in range(4):
                        for qt in range(2):
                            for grp in range(2 * (qt + 1)):
                                p8_kh0[(j, qt, grp)] = sc_exp(
                                    0, j, qt, grp, "p8a", 24)

                    for kh in range(NKV):
                        for j in range(4):
                            h = 4 * kh + j
                            for qt in range(2):
                                u = 2 * j + qt
                                nk = 8 * (qt + 1)
                                y_ps = psB.tile([128, CH], f32, tag="y",
                                                bufs=2)
                                den_ps = psB.tile([16, CH], f32, tag="den",
                                                  bufs=2)
                                if kh == 0:
                                    p8s = [p8_kh0[(j, qt, g)]
                                           for g in range(nk // 4)]
                                else:
                                    p8s = [sc_exp(kh, j, qt, g, "p8", 6)
                                           for g in range(nk // 4)]
                                y_den(kh, p8s, qt, y_ps, den_ps)
                                yraw = ap.tile([128, CH], bf16, tag="yraw",
                                               bufs=4, name="yraw")
                                nc.vector.tensor_copy(out=yraw[:],
                                                      in_=y_ps[:])
                                drow = wpool.tile([1, CH], f32, tag="srow",
                                                  bufs=3, name="drow")
                                nc.vector.tensor_copy(out=drow[:],
                                                      in_=den_ps[0:1, :])
                                # den carried 1/(16*8) so recip = 8/den
                                dre = wpool.tile([1, CH], f32, tag="srow",
                                                 bufs=3, name="dre")
                                nc.vector.reciprocal_approx_fast(dre[:],
                                                                 drow[:])
                                db = wpool.tile([128, CH], f32, tag="dbcast",
                                                bufs=2)
                                nc.gpsimd.partition_broadcast(db[:], dre[:])
                                nc.vector.tensor_tensor(
                                    yT8[:, h, CH * qt:CH * (qt + 1)],
                                    yraw[:], db[:], Alu.mult)
                ap_cm.__exit__(None, None, None)

                # ---- wo projection (fp8 DR) + residual (feature-major) ----
                with tc.tile_pool(name="psC", bufs=1, space="PSUM") as psC:
                    for n4 in range(4):
                        wof = wslab2(wo, NT * TQ * n4, TQ, NT, f"wo{n4}", fp8)
                        for k in range(4):
                            att_ps = psC.tile([128, TQ], f32, tag="att",
                                              bufs=4)
                            for hp in range(0, NH, 2):
                                nc.tensor.matmul(
                                    att_ps[:],
                                    lhsT=wof(hp, 128 * k, 128 * (k + 1),
                                             pair=True),
                                    rhs=yT8[:, hp:hp + 2, :],
                                    start=(hp == 0), stop=(hp == NH - 2),
                                    perf_mode=DR)
                            att_sb = wpool.tile([128, TQ], f32, tag="attsb",
                                                bufs=3)
                            # wo path carries 64 (w) * 8 (yT) = 512x
                            nc.scalar.activation(att_sb[:], att_ps[:],
                                                 Act.Copy, scale=1.0 / 512.0)
                            n = 4 * n4 + k
                            nc.vector.tensor_tensor(xmT[:, n], att_sb[:],
                                                    xin[:, n], Alu.add)
            # main pool closed (frees attention SBUF)

            # ---- MLP (bf16) ----
            with tc.tile_pool(name="mlp", bufs=1) as mlpp:
                h2T = mlpp.tile([128, NT, TQ], bf16, tag="h2T")
                a_sb = mlpp.tile([128, NF, TQ], bf16, tag="a_sb")

                with tc.tile_pool(name="psC2", bufs=1, space="PSUM") as psC2:
                    ssq2 = psC2.tile([1, TQ], f32, tag="row", bufs=2)
                    for i in range(NT):
                        xsq = wpool.tile([128, TQ], bf16, tag="xsq", bufs=6)
                        nc.vector.tensor_tensor(xsq[:], xmT[:, i], xmT[:, i],
                                                Alu.mult)
                        nc.tensor.matmul(ssq2[:], lhsT=ones[:], rhs=xsq[:],
                                         start=(i == 0), stop=(i == NT - 1))
                    s2b = norm_row(ssq2, 1.0 / C, EPS, TQ)
                    for i in range(NT):
                        nc.vector.tensor_tensor(h2T[:, i], xmT[:, i], s2b[:],
                                                Alu.mult)

                # fc + relu^2 (feature-major a)
                with tc.tile_pool(name="psD", bufs=1, space="PSUM") as psD:
                    for jc in range(16):
                        wfcf = wslab2(wfc, NT * TQ * jc, TQ, NT, f"wfc{jc}",
                                      bf16)
                        for jf in range(4):
                            f_ps = psD.tile([128, TQ], f32, tag="f", bufs=6)
                            for i in range(NT):
                                nc.tensor.matmul(
                                    f_ps[:],
                                    lhsT=wfcf(i, 128 * jf, 128 * (jf + 1)),
                                    rhs=h2T[:, i],
                                    start=(i == 0), stop=(i == NT - 1))
                            f = 4 * jc + jf
                            r_bf = wpool.tile([128, TQ], bf16, tag="r_bf")
                            nc.scalar.activation(r_bf[:], f_ps[:], Act.Relu)
                            nc.vector.tensor_tensor(a_sb[:, f], r_bf[:],
                                                    r_bf[:], Alu.mult)

                # proj: weight-stationary, feature-major output + residual
                with tc.tile_pool(name="psE", bufs=1, space="PSUM") as psE:
                    for n in range(16):
                        wpf = wslab2(wproj, NF * 128 * n, 128, NF, f"wp{n}",
                                     bf16)
                        o_ps = psE.tile([128, TQ], f32, tag="o", bufs=3)
                        for f in range(NF):
                            nc.tensor.matmul(o_ps[:], lhsT=wpf(f),
                                             rhs=a_sb[:, f, :],
                                             start=(f == 0),
                                             stop=(f == NF - 1))
                        ov = wpool.tile([128, TQ], f32, tag="ov", bufs=3)
                        nc.vector.tensor_tensor(ov[:], o_ps[:], xmT[:, n],
                                                Alu.add)
                        nc.sync.dma_start(out_fm[128 * n:128 * (n + 1), :],
                                          ov[:])

    nc.compile()
    return nc


def _prep_weights(wq, wk, wv, wo, w_fc, w_proj):
    import ml_dtypes
    bf = ml_dtypes.bfloat16
    f8 = ml_dtypes.float8_e4m3fn

    def tile_w(w, chunk, dtype, scale=1.0):
        # [R, F] -> [128, (F//chunk) * (R//128) * chunk]
        R, F = w.shape
        t = np.asarray(w, np.float32) * scale
        t = t.reshape(R // 128, 128, F // chunk, chunk)
        t = t.transpose(1, 2, 0, 3).reshape(128, -1)
        if dtype is f8:
            t = np.clip(t, -448.0, 448.0)
        return np.ascontiguousarray(t).astype(dtype)

    return {
        "wq": tile_w(wq, TQ, f8, SW),
        "wk": tile_w(wk, NKV * HD, f8, SW),
        "wv": tile_w(wv, NKV * HD, f8, SW),
        "wo": tile_w(wo, TQ, f8, SW),
        "wfc": tile_w(w_fc, TQ, bf),
        "wproj": tile_w(w_proj, 128, bf),
    }


def _make_in_maps(x, cos, sin, weights_b):
    import ml_dtypes
    bf = ml_dtypes.bfloat16
    f8 = ml_dtypes.float8_e4m3fn
    cosT = cos[0, :, 0, :].T  # [64, T]
    sinT = sin[0, :, 0, :].T
    kk = np.arange(128)
    qq = np.arange(CH)
    dd = np.arange(8)                           # band tile: d = dd//4, gp = dd%4
    in_maps = []
    for c in range(NCORES):
        b, g = divmod(c, 4)
        idx = 4 * np.arange(TQ) + g             # own token positions
        # key pos = 512*(2qt+d) + 4k + gp ; query pos = 1024qt + 4q + g
        off = 128 * (dd // 4) + (dd % 4 > g)
        msk = (qq[None, None, :] - kk[:, None, None]
               >= off[None, :, None]).astype(np.float32)
        mb = -30.0 * (1.0 - msk)
        xT = np.ascontiguousarray(x[b, idx, :].T)       # [C, TQ]
        xpt = xT.reshape(NT, 128, TQ).transpose(1, 0, 2).reshape(128, -1)
        m = {
            "xp": np.ascontiguousarray(xpt).astype(bf),
            "x8p": np.clip(np.ascontiguousarray(xpt), -448.0,
                           448.0).astype(f8),
            "csc": np.ascontiguousarray(
                np.concatenate([cosT[:, idx], cosT[:, idx]],
                               axis=0)).astype(bf),
            "css": np.ascontiguousarray(
                np.concatenate([sinT[:, idx], -sinT[:, idx]],
                               axis=0)).astype(bf),
            "mbias": np.ascontiguousarray(mb.reshape(128, 8 * CH)).astype(bf),
            "ident": np.ascontiguousarray(
                np.eye(128, dtype=np.float32)).astype(bf),
        }
        m.update(weights_b)
        in_maps.append(m)
    return in_maps


def kernel(x, cos, sin, wq, wk, wv, wo, w_fc, w_proj):
    global _CACHE
    from concourse.bass_utils import run_bass_kernel_spmd

    x = np.asarray(x, np.float32)
    cos = np.asarray(cos, np.float32)
    sin = np.asarray(sin, np.float32)
    weights_b = _prep_weights(wq, wk, wv, wo, w_fc, w_proj)

    if _CACHE is None:
        _CACHE = _build()
    nc = _CACHE

    in_maps = _make_in_maps(x, cos, sin, weights_b)
    res = run_bass_kernel_spmd(nc, in_maps, list(range(NCORES)))
    out = np.empty((B, T, C), np.float32)
    for c in range(NCORES):
        b, g = divmod(c, 4)
        idx = 4 * np.arange(TQ) + g
        out[b, idx, :] = res.results[c]["out"].T
    return out
